# revision 9
# baseline (speedup 1.0000x reference)
"""Trainium2 Bass kernel for nn_AttentionBlockV2 (dense transformer block).

Sharding: 8 cores; core c handles batch b=c//4, image row-block r=c%4
(12 of 48 rows = 576 query pixels). Each core computes q/k/v for its FULL
batch (2304 keys; spatially rotated so the program is uniform across cores),
runs attention for its 576 queries over all keys, then the depthwise pos-enc
conv, projection, residuals and the conv-FFN for its local pixels.
No collectives: the host reassembles the 8 local outputs.

Self-contained: hardcodes all shapes; imports only numpy/ml_dtypes/concourse.
"""
import sys
import numpy as np
import ml_dtypes

try:
    import concourse.bass  # noqa: F401
except ImportError:  # fallback when the axon site path isn't preloaded
    sys.path.insert(0, "/opt/trn_rl_repo")

import bass_rust
import concourse.bass as bass
from concourse import bacc
import concourse.mybir as mybir
import concourse.tile as tile
from concourse.vector_clock import ScopedClock
from concourse.bass_utils import run_bass_kernel_spmd

BF16 = ml_dtypes.bfloat16
DT = mybir.dt.bfloat16
F32 = mybir.dt.float32
AF = mybir.ActivationFunctionType
ALU = mybir.AluOpType

# problem constants
B, C, NH, HD, KK, H1 = 2, 256, 8, 32, 7, 512
HS = WS = 48
N = HS * WS            # 2304 keys per batch
NQ = 576               # local queries per core
CH = 288               # query chunk (2 per core)
NKT = N // 128         # 18 key tiles
SCALE = HD ** -0.5
N_CORES = 8

# bias row layout in the packed [16, 128] bias tensor
BIAS_QKV = 0    # rows 0-5: q0,q1,k0,k1,v0,v1 (also v_b for o-norm at rows 4-5)
BIAS_PROJ = 6   # rows 6-7
BIAS_FC1 = 8    # rows 8-11
BIAS_FC2 = 12   # rows 12-13
BIAS_PE = 14    # rows 14-15


def _patched_drain_and_barrier(self, tick_clock, wait_clock):
    # upstream emits one epilogue drain carrying every outstanding wait;
    # walrus codegen accepts at most one sync wait per CTRL instruction,
    # so spread the extras over additional drains.
    drain_inst = self.nc.sync.drain()
    wait_clock.add_sem_waits(drain_inst.ins, ScopedClock({None: tick_clock.global_clock}))
    si = drain_inst.ins.sync_info
    waits = list(si.on_wait) if si is not None else []
    if len(waits) > 1:
        si.on_wait = [waits[0]]
        drain_inst.ins.sync_info = si
        for w in waits[1:]:
            extra = self.nc.sync.drain()
            extra.ins.sync_info = bass_rust.SyncInfo(on_wait=[w], on_update=[])
    self.nc.all_engine_barrier()
    assert self.sems is not None
    popped = self.nc._tile_sem_poison_stack.pop()
    assert popped is self._sem_poison
    self.nc.clear_and_free_semaphores(list(self.sems.allocated().values()))
    self.nc.all_engine_barrier()


tile.TileContext._drain_and_barrier = _patched_drain_and_barrier

# taps executed on the vector engine (rest go to gpsimd); tuned for balance
N_DVE_TAPS = 37


def build_kernel():
    from contextlib import ExitStack

    nc = bacc.Bacc("TRN2", target_bir_lowering=False, debug=False)
    ap_xb = nc.dram_tensor("xb", (2, 128, N), DT, kind="ExternalInput").ap()
    ap_xloc = nc.dram_tensor("xloc", (2, 128, NQ), F32, kind="ExternalInput").ap()
    ap_pemask = nc.dram_tensor("pemask", (128, 18 * 54), DT, kind="ExternalInput").ap()
    ap_wqkv = nc.dram_tensor("wqkv", (2, 128, 768), DT, kind="ExternalInput").ap()
    ap_wvt = nc.dram_tensor("wvt", (2, 128, 256), DT, kind="ExternalInput").ap()
    ap_wproj = nc.dram_tensor("wproj", (2, 128, 256), DT, kind="ExternalInput").ap()
    ap_wfc1 = nc.dram_tensor("wfc1", (2, 128, 512), DT, kind="ExternalInput").ap()
    ap_wfc2 = nc.dram_tensor("wfc2", (4, 128, 256), DT, kind="ExternalInput").ap()
    ap_pew = nc.dram_tensor("pew", (2, 128, 49), F32, kind="ExternalInput").ap()
    ap_bias = nc.dram_tensor("bias", (16, 128), F32, kind="ExternalInput").ap()
    ap_y = nc.dram_tensor("y", (2, 128, NQ), F32, kind="ExternalOutput").ap()

    with tile.TileContext(nc) as tc, ExitStack() as ctx:
        const = ctx.enter_context(tc.tile_pool(name="const", bufs=1))
        persist = ctx.enter_context(tc.tile_pool(name="persist", bufs=1))
        work = ctx.enter_context(tc.tile_pool(name="work", bufs=2))
        ppool = ctx.enter_context(tc.tile_pool(name="pbuf", bufs=2))
        psum = ctx.enter_context(tc.tile_pool(name="psum", bufs=2, space="PSUM"))

        # ---- loads ----
        w_qkv = const.tile([128, 2, 768], DT)
        nc.sync.dma_start(w_qkv[:], ap_wqkv.rearrange("a p m -> p a m"))
        w_vt = const.tile([128, 2, 256], DT)
        nc.sync.dma_start(w_vt[:], ap_wvt.rearrange("a p m -> p a m"))
        w_proj = const.tile([128, 2, 256], DT)
        nc.sync.dma_start(w_proj[:], ap_wproj.rearrange("a p m -> p a m"))
        w_fc1 = const.tile([128, 2, 512], DT)
        nc.sync.dma_start(w_fc1[:], ap_wfc1.rearrange("a p m -> p a m"))
        w_fc2 = const.tile([128, 4, 256], DT)
        nc.sync.dma_start(w_fc2[:], ap_wfc2.rearrange("a p m -> p a m"))
        pew = const.tile([128, 2, 49], F32)
        nc.sync.dma_start(pew[:], ap_pew.rearrange("a p m -> p a m"))
        bias = const.tile([128, 16], F32)
        nc.sync.dma_start(bias[:], ap_bias.rearrange("a p -> p a"))
        pemask = const.tile([128, 18, 54], DT)
        nc.sync.dma_start(pemask[:], ap_pemask.rearrange("p (a b) -> p a b", a=18))
        ones_sb = const.tile([128, 32], DT)
        nc.vector.memset(ones_sb[:], 1.0)

        NCH = [(0, 512), (512, 512), (1024, 512), (1536, 512), (2048, 256)]
        x_sb = persist.tile([128, 2, N], DT)
        for c0, cw in NCH:   # chunked so convs start before the full x lands
            nc.sync.dma_start(x_sb[:, :, c0:c0 + cw], ap_xb.rearrange("a p n -> p a n")[:, :, c0:c0 + cw])
        xloc = persist.tile([128, 2, NQ], F32)
        nc.sync.dma_start(xloc[:], ap_xloc.rearrange("a p n -> p a n"))

        # ---- prologue: k conv chunk0 + q conv (rest of k via work queue) ----
        k_sb = persist.tile([128, 2, N], DT)

        def emit_kconv(arg):
            c0, cw = arg
            for mt in range(2):
                ps = psum.tile([128, 4, 512], F32, tag="ps", name="ps_k")
                for kt in range(2):
                    nc.tensor.matmul(
                        ps[:, 0, :cw],
                        w_qkv[:, kt, 128 * (2 + mt):128 * (3 + mt)],
                        x_sb[:, kt, c0:c0 + cw],
                        start=(kt == 0), stop=(kt == 1))
                nc.scalar.activation(k_sb[:, mt, c0:c0 + cw], ps[:, 0, :cw],
                                     AF.Identity, bias=bias[:, 2 + mt:3 + mt])

        emit_kconv(NCH[0])
        q_sb = persist.tile([128, 2, NQ], DT)
        for mt in range(2):
            for c0, cw in [(0, 512), (512, 64)]:
                ps = psum.tile([128, 4, 512], F32, tag="ps", name="ps_q")
                for kt in range(2):
                    nc.tensor.matmul(
                        ps[:, 0, :cw],
                        w_qkv[:, kt, 128 * mt:128 * (mt + 1)],
                        x_sb[:, kt, c0:c0 + cw],
                        start=(kt == 0), stop=(kt == 1))
                nc.scalar.activation(q_sb[:, mt, c0:c0 + cw], ps[:, 0, :cw],
                                     AF.Identity, bias=bias[:, mt:mt + 1])

        # persistent tiles used by interleaved work
        vt = persist.tile([128, NKT, 256], DT)
        vpad = persist.tile([128, 2, 18, 54], DT)
        vpad1 = persist.tile([128, 2, 18, 54], DT)
        pe_a = persist.tile([128, 2, 12, 48], DT)
        pe_b = persist.tile([128, 2, 12, 48], DT)
        proj_in = persist.tile([128, 2, NQ], DT)
        x1 = persist.tile([128, 2, NQ], F32)
        x1b = persist.tile([128, 2, NQ], DT)
        h_sb = persist.tile([128, 4, NQ], DT)
        y_sb = persist.tile([128, 2, NQ], F32)
        nc.gpsimd.memset(vpad[:], 0.0)

        VW = [(2160, 144, 0, 3), (0, 480, 3, 10), (480, 240, 13, 5)]

        def emit_vconv(job):
            ct, (c0, cw, r0, nr) = job
            ps = psum.tile([128, 4, 512], F32, tag="ps", name="ps_v")
            for kt in range(2):
                nc.tensor.matmul(
                    ps[:, 0, :cw],
                    w_qkv[:, kt, 128 * (4 + ct):128 * (5 + ct)],
                    x_sb[:, kt, c0:c0 + cw],
                    start=(kt == 0), stop=(kt == 1))
            nc.vector.scalar_tensor_tensor(
                vpad[:, ct, r0:r0 + nr, 3:51],
                ps[:, 0, :cw].rearrange("p (a b) -> p a b", b=48),
                bias[:, 4 + ct:5 + ct],
                pemask[:, r0:r0 + nr, 3:51],
                ALU.add, ALU.mult)

        def emit_vpad1(_):
            nc.vector.tensor_copy(vpad1[:, :, :, 0:53], vpad[:, :, :, 1:54])

        def emit_vt(mt):
            ps = psum.tile([128, 4, 512], F32, tag="ps", name="ps_vt")
            for kt in range(2):
                nc.tensor.matmul(
                    ps[:, 0, :256],
                    x_sb[:, kt, 128 * mt:128 * (mt + 1)],
                    w_vt[:, kt, :],
                    start=(kt == 0), stop=(kt == 1))
            nc.vector.tensor_copy(vt[:, mt, :], ps[:, 0, :256])

        pe_b_started = [False, False]

        def emit_tap(job):
            kind, ct, dy, dx, first = job
            if dx % 2 == 0:
                win = vpad[:, ct, dy:dy + 12, dx:dx + 48]
            else:
                win = vpad1[:, ct, dy:dy + 12, dx - 1:dx + 47]
            sc = pew[:, ct, dy * 7 + dx:dy * 7 + dx + 1]
            if kind == "dve":
                if first:
                    nc.vector.tensor_scalar(pe_a[:, ct], win, sc,
                                            bias[:, BIAS_PE + ct:BIAS_PE + ct + 1],
                                            ALU.mult, ALU.add)
                else:
                    nc.vector.scalar_tensor_tensor(pe_a[:, ct], win, sc,
                                                   pe_a[:, ct], ALU.mult, ALU.add)
            else:
                tmp = work.tile([128, 12, 48], DT, tag="ttmp", name="ttmp")
                nc.vector.tensor_scalar_mul(tmp[:], win, sc)
                if not pe_b_started[ct]:
                    pe_b_started[ct] = True
                    nc.gpsimd.tensor_copy(pe_b[:, ct], tmp[:])
                else:
                    nc.gpsimd.tensor_tensor(pe_b[:, ct], pe_b[:, ct], tmp[:], ALU.add)

        def emit_pe_merge(ct):
            nc.gpsimd.tensor_tensor(pe_a[:, ct], pe_a[:, ct], pe_b[:, ct], ALU.add)

        # build the paced work queue
        taps = [(dy, dx) for dy in range(7) for dx in range(7)]
        queue = []
        for ch in NCH[1:]:
            queue.append((emit_kconv, ch))
        for ct in range(2):
            for vw in VW:
                queue.append((emit_vconv, (ct, vw)))
        queue.append((emit_vpad1, None))
        for mt in range(NKT):
            queue.append((emit_vt, mt))
        phase1_len = len(queue)           # consumed during att(0,0)
        for ct in range(2):
            for i, (dy, dx) in enumerate(taps[:29]):
                queue.append((emit_tap, ("dve", ct, dy, dx, i == 0)))
            for dy, dx in taps[29:]:
                queue.append((emit_tap, ("gps", ct, dy, dx, False)))
        for ct in range(2):
            queue.append((emit_pe_merge, ct))
        qpos = [0]

        def consume(n):
            lim = min(qpos[0] + n, len(queue))
            while qpos[0] < lim:
                fn, arg = queue[qpos[0]]
                qpos[0] += 1
                fn(arg)

        def emit_attention(g, c, per_mt, pre_osum=None):
            pt = ppool.tile([128, 4, NKT, CH], DT, tag="P", name=f"P_{g}_{c}")
            for mt in range(NKT):
                sc = psum.tile([128, 4, 512], F32, tag="ps", name="ps_sc")
                for h in range(4):
                    nc.tensor.matmul(
                        sc[:, h, :CH],
                        k_sb[32 * h:32 * h + 32, g, 128 * mt:128 * (mt + 1)],
                        q_sb[32 * h:32 * h + 32, g, CH * c:CH * (c + 1)],
                        start=True, stop=True, tile_position=(32 * h, 0))
                nc.scalar.activation(pt[:, :, mt, :], sc[:, :, :CH], AF.Exp,
                                     scale=SCALE)
                consume(per_mt)
            if pre_osum is not None:
                pre_osum()
            osum = psum.tile([128, 4, 512], F32, tag="ps", name="ps_osum")
            for kt in range(NKT):
                for h in range(4):
                    nc.tensor.matmul(
                        osum[32 * h:32 * h + 32, 0, :CH],
                        vt[:, kt, 128 * g + 32 * h:128 * g + 32 * h + 32],
                        pt[:, h, kt, :],
                        start=(kt == 0), stop=(kt == NKT - 1),
                        tile_position=(0, 32 * h))
                for h in range(4):
                    nc.tensor.matmul(
                        osum[32 * h:32 * h + 32, 1, :CH],
                        ones_sb[:, 0:32],
                        pt[:, h, kt, :],
                        start=(kt == 0), stop=(kt == NKT - 1),
                        tile_position=(0, 32 * h))
            r_sb = work.tile([128, CH], F32, tag="recip", name="r_sb")
            nc.vector.reciprocal(r_sb[:], osum[:, 1, :CH])
            o_tmp = work.tile([128, CH], DT, tag="otmp", name="o_tmp")
            nc.vector.tensor_tensor(o_tmp[:], osum[:, 0, :CH], r_sb[:], ALU.mult)
            nc.vector.tensor_scalar_add(proj_in[:, g, CH * c:CH * (c + 1)],
                                        o_tmp[:], bias[:, 4 + g:5 + g])

        def emit_ffn(c):
            for g in range(2):
                sl = slice(CH * c, CH * (c + 1))
                nc.gpsimd.tensor_tensor(
                    proj_in[:, g, sl], proj_in[:, g, sl],
                    pe_a[:, g].rearrange("p a b -> p (a b)")[:, sl], ALU.add)
            for mt in range(2):
                ps = psum.tile([128, 4, 512], F32, tag="ps", name="ps_proj")
                for kt in range(2):
                    nc.tensor.matmul(
                        ps[:, 0, :CH],
                        w_proj[:, kt, 128 * mt:128 * (mt + 1)],
                        proj_in[:, kt, CH * c:CH * (c + 1)],
                        start=(kt == 0), stop=(kt == 1))
                nc.vector.scalar_tensor_tensor(
                    x1[:, mt, CH * c:CH * (c + 1)], ps[:, 0, :CH],
                    bias[:, BIAS_PROJ + mt:BIAS_PROJ + mt + 1],
                    xloc[:, mt, CH * c:CH * (c + 1)], ALU.add, ALU.add)
                nc.gpsimd.tensor_copy(x1b[:, mt, CH * c:CH * (c + 1)],
                                      x1[:, mt, CH * c:CH * (c + 1)])
            for mt in range(4):
                ps = psum.tile([128, 4, 512], F32, tag="ps", name="ps_fc1")
                for kt in range(2):
                    nc.tensor.matmul(
                        ps[:, 0, :CH],
                        w_fc1[:, kt, 128 * mt:128 * (mt + 1)],
                        x1b[:, kt, CH * c:CH * (c + 1)],
                        start=(kt == 0), stop=(kt == 1))
                nc.scalar.activation(h_sb[:, mt, CH * c:CH * (c + 1)], ps[:, 0, :CH],
                                     AF.Silu, bias=bias[:, BIAS_FC1 + mt:BIAS_FC1 + mt + 1])
            for mt in range(2):
                ps = psum.tile([128, 4, 512], F32, tag="ps", name="ps_fc2")
                for kt in range(4):
                    nc.tensor.matmul(
                        ps[:, 0, :CH],
                        w_fc2[:, kt, 128 * mt:128 * (mt + 1)],
                        h_sb[:, kt, CH * c:CH * (c + 1)],
                        start=(kt == 0), stop=(kt == 3))
                nc.vector.scalar_tensor_tensor(
                    y_sb[:, mt, CH * c:CH * (c + 1)], ps[:, 0, :CH],
                    bias[:, BIAS_FC2 + mt:BIAS_FC2 + mt + 1],
                    x1[:, mt, CH * c:CH * (c + 1)], ALU.add, ALU.add)
                nc.sync.dma_start(ap_y[mt, :, CH * c:CH * (c + 1)],
                                  y_sb[:, mt, CH * c:CH * (c + 1)])

        # ---- main pipeline ----
        emit_attention(0, 0, per_mt=2)          # consumes convs/vT
        emit_attention(1, 0, per_mt=3)          # consumes taps
        emit_attention(0, 1, per_mt=3)
        emit_attention(1, 1, per_mt=2, pre_osum=lambda: (consume(len(queue)), emit_ffn(0)))
        emit_ffn(1)
    nc.compile()
    return nc


_CACHED = {}


def _get_nc():
    if "nc" not in _CACHED:
        _CACHED["nc"] = build_kernel()
    return _CACHED["nc"]


def _prep_inputs(inputs):
    x = np.asarray(inputs["x"], np.float32)           # [2, 256, 48, 48]
    qk_w = np.asarray(inputs["qk_w"], np.float32)
    qk_b = np.asarray(inputs["qk_b"], np.float32)
    v_w = np.asarray(inputs["v_w"], np.float32)
    v_b = np.asarray(inputs["v_b"], np.float32)
    pe_w = np.asarray(inputs["pe_w"], np.float32)
    pe_b = np.asarray(inputs["pe_b"], np.float32)
    proj_w = np.asarray(inputs["proj_w"], np.float32)
    proj_b = np.asarray(inputs["proj_b"], np.float32)
    fc1_w = np.asarray(inputs["fc1_w"], np.float32)
    fc1_b = np.asarray(inputs["fc1_b"], np.float32)
    fc2_w = np.asarray(inputs["fc2_w"], np.float32)
    fc2_b = np.asarray(inputs["fc2_b"], np.float32)

    rows = np.arange(2 * C).reshape(NH, 2, HD)
    q_rows = rows[:, 0, :].reshape(-1)
    k_rows = rows[:, 1, :].reshape(-1)
    w_qkv = np.concatenate([qk_w[q_rows], qk_w[k_rows], v_w], axis=0)  # [768, 256]
    wqkv = np.ascontiguousarray(w_qkv.T.reshape(2, 128, 768)).astype(BF16)
    wvt = np.ascontiguousarray(v_w.T.reshape(2, 128, 256)).astype(BF16)
    wproj = np.ascontiguousarray(proj_w.T.reshape(2, 128, 256)).astype(BF16)
    wfc1 = np.ascontiguousarray(fc1_w.T.reshape(2, 128, 512)).astype(BF16)
    wfc2 = np.ascontiguousarray(fc2_w.T.reshape(4, 128, 256)).astype(BF16)
    pew = np.ascontiguousarray(pe_w[:, 0].reshape(2, 128, 49)).astype(np.float32)

    bias = np.zeros((16, 128), np.float32)
    bias[0:2] = qk_b[q_rows].reshape(2, 128)
    bias[2:4] = qk_b[k_rows].reshape(2, 128)
    bias[4:6] = v_b.reshape(2, 128)
    bias[6:8] = proj_b.reshape(2, 128)
    bias[8:12] = fc1_b.reshape(4, 128)
    bias[12:14] = fc2_b.reshape(2, 128)
    bias[14:16] = pe_b.reshape(2, 128)

    xn = x.reshape(B, C, HS, WS)
    in_maps = []
    for core in range(N_CORES):
        b, r = core // 4, core % 4
        xrot = np.roll(xn[b], -12 * r, axis=1)                 # rotate rows
        xb = np.ascontiguousarray(xrot.reshape(C, N).reshape(2, 128, N)).astype(BF16)
        xloc = np.ascontiguousarray(
            xrot[:, :12, :].reshape(C, NQ).reshape(2, 128, NQ)).astype(np.float32)
        mask = np.ones((18, 54), np.float32)
        if r == 0:
            mask[0:3, :] = 0.0                                  # top image border
        if r == 3:
            mask[15:18, :] = 0.0                                # bottom image border
        pemask = np.broadcast_to(mask.reshape(1, 972), (128, 972)).astype(BF16)
        in_maps.append({
            "xb": xb, "xloc": xloc, "pemask": np.ascontiguousarray(pemask),
            "wqkv": wqkv, "wvt": wvt, "wproj": wproj, "wfc1": wfc1, "wfc2": wfc2,
            "pew": pew, "bias": bias,
        })
    return in_maps


def kernel(**inputs) -> np.ndarray:
    nc = _get_nc()
    in_maps = _prep_inputs(inputs)
    res = run_bass_kernel_spmd(nc, in_maps, core_ids=list(range(N_CORES)),
                               trace=False)
    out = np.zeros((B, C, HS, WS), np.float32)
    for core in range(N_CORES):
        b, r = core // 4, core % 4
        y = res.results[core]["y"].reshape(C, 12, WS)
        out[b, :, 12 * r:12 * (r + 1), :] = y
    return out


def run_traced(inputs):
    """test-harness helper: run with NTFF tracing, return (out, results)."""
    nc = _get_nc()
    in_maps = _prep_inputs(inputs)
    res = run_bass_kernel_spmd(nc, in_maps, core_ids=list(range(N_CORES)),
                               trace=True)
    out = np.zeros((B, C, HS, WS), np.float32)
    for core in range(N_CORES):
        b, r = core // 4, core % 4
        y = res.results[core]["y"].reshape(C, 12, WS)
        out[b, :, 12 * r:12 * (r + 1), :] = y
    return out, res


# revision 10
# speedup vs baseline: 1.0152x; 1.0152x over previous
"""Trainium2 Bass kernel for nn_AttentionBlockV2 (dense transformer block).

Sharding: 8 cores; core c handles batch b=c//4, image row-block r=c%4
(12 of 48 rows = 576 query pixels). Each core computes q/k/v for its FULL
batch (2304 keys; spatially rotated so the program is uniform across cores),
runs attention for its 576 queries over all keys, then the depthwise pos-enc
conv, projection, residuals and the conv-FFN for its local pixels.
No collectives: the host reassembles the 8 local outputs.

Self-contained: hardcodes all shapes; imports only numpy/ml_dtypes/concourse.
"""
import sys
import numpy as np
import ml_dtypes

try:
    import concourse.bass  # noqa: F401
except ImportError:  # fallback when the axon site path isn't preloaded
    sys.path.insert(0, "/opt/trn_rl_repo")

import bass_rust
import concourse.bass as bass
from concourse import bacc
import concourse.mybir as mybir
import concourse.tile as tile
from concourse.tile import add_dep_helper
from concourse.vector_clock import ScopedClock
from concourse.bass_utils import run_bass_kernel_spmd

BF16 = ml_dtypes.bfloat16
DT = mybir.dt.bfloat16
F32 = mybir.dt.float32
AF = mybir.ActivationFunctionType
ALU = mybir.AluOpType

# problem constants
B, C, NH, HD, KK, H1 = 2, 256, 8, 32, 7, 512
HS = WS = 48
N = HS * WS            # 2304 keys per batch
NQ = 576               # local queries per core
CH = 288               # query chunk (2 per core)
NKT = N // 128         # 18 key tiles
SCALE = HD ** -0.5
N_CORES = 8

# bias row layout in the packed [16, 128] bias tensor
BIAS_QKV = 0    # rows 0-5: q0,q1,k0,k1,v0,v1 (also v_b for o-norm at rows 4-5)
BIAS_PROJ = 6   # rows 6-7
BIAS_FC1 = 8    # rows 8-11
BIAS_FC2 = 12   # rows 12-13
BIAS_PE = 14    # rows 14-15


def _patched_drain_and_barrier(self, tick_clock, wait_clock):
    # upstream emits one epilogue drain carrying every outstanding wait;
    # walrus codegen accepts at most one sync wait per CTRL instruction,
    # so spread the extras over additional drains.
    drain_inst = self.nc.sync.drain()
    wait_clock.add_sem_waits(drain_inst.ins, ScopedClock({None: tick_clock.global_clock}))
    si = drain_inst.ins.sync_info
    waits = list(si.on_wait) if si is not None else []
    if len(waits) > 1:
        si.on_wait = [waits[0]]
        drain_inst.ins.sync_info = si
        for w in waits[1:]:
            extra = self.nc.sync.drain()
            extra.ins.sync_info = bass_rust.SyncInfo(on_wait=[w], on_update=[])
    self.nc.all_engine_barrier()
    assert self.sems is not None
    popped = self.nc._tile_sem_poison_stack.pop()
    assert popped is self._sem_poison
    self.nc.clear_and_free_semaphores(list(self.sems.allocated().values()))
    self.nc.all_engine_barrier()


tile.TileContext._drain_and_barrier = _patched_drain_and_barrier

# taps executed on the vector engine (rest go to gpsimd); tuned for balance
N_DVE_TAPS = 37


def build_kernel():
    from contextlib import ExitStack

    nc = bacc.Bacc("TRN2", target_bir_lowering=False, debug=False)
    ap_xb = nc.dram_tensor("xb", (2, 128, N), DT, kind="ExternalInput").ap()
    ap_xloc = nc.dram_tensor("xloc", (2, 128, NQ), F32, kind="ExternalInput").ap()
    ap_pemask = nc.dram_tensor("pemask", (128, 18 * 54), DT, kind="ExternalInput").ap()
    ap_wqkv = nc.dram_tensor("wqkv", (2, 128, 768), DT, kind="ExternalInput").ap()
    ap_wvt = nc.dram_tensor("wvt", (2, 128, 256), DT, kind="ExternalInput").ap()
    ap_wproj = nc.dram_tensor("wproj", (2, 128, 256), DT, kind="ExternalInput").ap()
    ap_wfc1 = nc.dram_tensor("wfc1", (2, 128, 512), DT, kind="ExternalInput").ap()
    ap_wfc2 = nc.dram_tensor("wfc2", (4, 128, 256), DT, kind="ExternalInput").ap()
    ap_pew = nc.dram_tensor("pew", (2, 128, 49), F32, kind="ExternalInput").ap()
    ap_bias = nc.dram_tensor("bias", (16, 128), F32, kind="ExternalInput").ap()
    ap_y = nc.dram_tensor("y", (2, 128, NQ), F32, kind="ExternalOutput").ap()

    with tile.TileContext(nc) as tc, ExitStack() as ctx:
        const = ctx.enter_context(tc.tile_pool(name="const", bufs=1))
        persist = ctx.enter_context(tc.tile_pool(name="persist", bufs=1))
        work = ctx.enter_context(tc.tile_pool(name="work", bufs=2))
        ppool = ctx.enter_context(tc.tile_pool(name="pbuf", bufs=2))
        psum = ctx.enter_context(tc.tile_pool(name="psum", bufs=2, space="PSUM"))

        # ---- loads ----
        w_qkv = const.tile([128, 2, 768], DT)
        nc.sync.dma_start(w_qkv[:], ap_wqkv.rearrange("a p m -> p a m"))
        w_vt = const.tile([128, 2, 256], DT)
        nc.sync.dma_start(w_vt[:], ap_wvt.rearrange("a p m -> p a m"))
        w_proj = const.tile([128, 2, 256], DT)
        nc.sync.dma_start(w_proj[:], ap_wproj.rearrange("a p m -> p a m"))
        w_fc1 = const.tile([128, 2, 512], DT)
        nc.sync.dma_start(w_fc1[:], ap_wfc1.rearrange("a p m -> p a m"))
        w_fc2 = const.tile([128, 4, 256], DT)
        nc.sync.dma_start(w_fc2[:], ap_wfc2.rearrange("a p m -> p a m"))
        pew = const.tile([128, 2, 49], F32)
        nc.sync.dma_start(pew[:], ap_pew.rearrange("a p m -> p a m"))
        bias = const.tile([128, 16], F32)
        nc.sync.dma_start(bias[:], ap_bias.rearrange("a p -> p a"))
        pemask = const.tile([128, 18, 54], DT)
        nc.sync.dma_start(pemask[:], ap_pemask.rearrange("p (a b) -> p a b", a=18))
        ones_sb = const.tile([128, 32], DT)
        nc.vector.memset(ones_sb[:], 1.0)

        NCH = [(0, 512), (512, 512), (1024, 512), (1536, 512), (2048, 256)]
        x_sb = persist.tile([128, 2, N], DT)
        for c0, cw in NCH:   # chunked so convs start before the full x lands
            nc.sync.dma_start(x_sb[:, :, c0:c0 + cw], ap_xb.rearrange("a p n -> p a n")[:, :, c0:c0 + cw])
        xloc = persist.tile([128, 2, NQ], F32)
        nc.sync.dma_start(xloc[:], ap_xloc.rearrange("a p n -> p a n"))

        # ---- prologue: k conv chunk0 + q conv (rest of k via work queue) ----
        k_sb = persist.tile([128, 2, N], DT)

        def emit_kconv(arg):
            c0, cw = arg
            for mt in range(2):
                ps = psum.tile([128, 4, 512], F32, tag="ps", name="ps_k")
                for kt in range(2):
                    nc.tensor.matmul(
                        ps[:, 0, :cw],
                        w_qkv[:, kt, 128 * (2 + mt):128 * (3 + mt)],
                        x_sb[:, kt, c0:c0 + cw],
                        start=(kt == 0), stop=(kt == 1))
                nc.scalar.activation(k_sb[:, mt, c0:c0 + cw], ps[:, 0, :cw],
                                     AF.Identity, bias=bias[:, 2 + mt:3 + mt])

        emit_kconv(NCH[0])
        q_sb = persist.tile([128, 2, NQ], DT)
        for mt in range(2):
            for c0, cw in [(0, 512), (512, 64)]:
                ps = psum.tile([128, 4, 512], F32, tag="ps", name="ps_q")
                for kt in range(2):
                    nc.tensor.matmul(
                        ps[:, 0, :cw],
                        w_qkv[:, kt, 128 * mt:128 * (mt + 1)],
                        x_sb[:, kt, c0:c0 + cw],
                        start=(kt == 0), stop=(kt == 1))
                nc.scalar.activation(q_sb[:, mt, c0:c0 + cw], ps[:, 0, :cw],
                                     AF.Identity, bias=bias[:, mt:mt + 1])

        # persistent tiles used by interleaved work
        vt = persist.tile([128, NKT, 256], DT)
        vpad = persist.tile([128, 2, 18, 54], DT)
        vpad1 = persist.tile([128, 2, 18, 54], DT)
        pe_a = persist.tile([128, 2, 12, 48], DT)
        pe_b = persist.tile([128, 2, 12, 48], DT)
        proj_in = persist.tile([128, 2, NQ], DT)
        x1 = persist.tile([128, 2, NQ], F32)
        x1b = persist.tile([128, 2, NQ], DT)
        h_sb = persist.tile([128, 4, NQ], DT)
        y_sb = persist.tile([128, 2, NQ], F32)
        nc.gpsimd.memset(vpad[:], 0.0)

        VW = [(2160, 144, 0, 3), (0, 480, 3, 10), (480, 240, 13, 5)]

        def emit_vconv(job):
            ct, (c0, cw, r0, nr) = job
            ps = psum.tile([128, 4, 512], F32, tag="ps", name="ps_v")
            for kt in range(2):
                nc.tensor.matmul(
                    ps[:, 0, :cw],
                    w_qkv[:, kt, 128 * (4 + ct):128 * (5 + ct)],
                    x_sb[:, kt, c0:c0 + cw],
                    start=(kt == 0), stop=(kt == 1))
            nc.vector.scalar_tensor_tensor(
                vpad[:, ct, r0:r0 + nr, 3:51],
                ps[:, 0, :cw].rearrange("p (a b) -> p a b", b=48),
                bias[:, 4 + ct:5 + ct],
                pemask[:, r0:r0 + nr, 3:51],
                ALU.add, ALU.mult)

        def emit_vpad1(_):
            nc.vector.tensor_copy(vpad1[:, :, :, 0:53], vpad[:, :, :, 1:54])

        def emit_vt(mt):
            ps = psum.tile([128, 4, 512], F32, tag="ps", name="ps_vt")
            for kt in range(2):
                nc.tensor.matmul(
                    ps[:, 0, :256],
                    x_sb[:, kt, 128 * mt:128 * (mt + 1)],
                    w_vt[:, kt, :],
                    start=(kt == 0), stop=(kt == 1))
            nc.vector.tensor_copy(vt[:, mt, :], ps[:, 0, :256])

        pe_b_started = [False, False]
        gate = [None]

        def _gated(bi):
            if gate[0] is not None:
                add_dep_helper(bi.ins, gate[0].ins, sync=False,
                               reason="pace filler work to the exp cadence")
            return bi

        def emit_tap(job):
            kind, ct, dy, dx, first = job
            if dx % 2 == 0:
                win = vpad[:, ct, dy:dy + 12, dx:dx + 48]
            else:
                win = vpad1[:, ct, dy:dy + 12, dx - 1:dx + 47]
            sc = pew[:, ct, dy * 7 + dx:dy * 7 + dx + 1]
            if kind == "dve":
                if first:
                    _gated(nc.vector.tensor_scalar(pe_a[:, ct], win, sc,
                                                   bias[:, BIAS_PE + ct:BIAS_PE + ct + 1],
                                                   ALU.mult, ALU.add))
                else:
                    _gated(nc.vector.scalar_tensor_tensor(pe_a[:, ct], win, sc,
                                                          pe_a[:, ct], ALU.mult, ALU.add))
            else:
                tmp = work.tile([128, 12, 48], DT, tag="ttmp", name="ttmp")
                _gated(nc.vector.tensor_scalar_mul(tmp[:], win, sc))
                if not pe_b_started[ct]:
                    pe_b_started[ct] = True
                    _gated(nc.gpsimd.tensor_copy(pe_b[:, ct], tmp[:]))
                else:
                    _gated(nc.gpsimd.tensor_tensor(pe_b[:, ct], pe_b[:, ct], tmp[:], ALU.add))

        def emit_pe_merge(ct):
            _gated(nc.gpsimd.tensor_tensor(pe_a[:, ct], pe_a[:, ct], pe_b[:, ct], ALU.add))

        # build the paced work queue
        taps = [(dy, dx) for dy in range(7) for dx in range(7)]
        queue = []
        for ch in NCH[1:]:
            queue.append((emit_kconv, ch))
        for ct in range(2):
            for vw in VW:
                queue.append((emit_vconv, (ct, vw)))
        queue.append((emit_vpad1, None))
        for mt in range(NKT):
            queue.append((emit_vt, mt))
        phase1_len = len(queue)           # consumed during att(0,0)
        for ct in range(2):
            for i, (dy, dx) in enumerate(taps[:29]):
                queue.append((emit_tap, ("dve", ct, dy, dx, i == 0)))
            for dy, dx in taps[29:]:
                queue.append((emit_tap, ("gps", ct, dy, dx, False)))
        for ct in range(2):
            queue.append((emit_pe_merge, ct))
        qpos = [0]

        def consume(n):
            lim = min(qpos[0] + n, len(queue))
            while qpos[0] < lim:
                fn, arg = queue[qpos[0]]
                qpos[0] += 1
                fn(arg)

        def emit_attention(g, c, per_mt, pre_osum=None):
            pt = ppool.tile([128, 4, NKT, CH], DT, tag="P", name=f"P_{g}_{c}")
            for mt in range(NKT):
                sc = psum.tile([128, 4, 512], F32, tag="ps", name="ps_sc")
                for h in range(4):
                    nc.tensor.matmul(
                        sc[:, h, :CH],
                        k_sb[32 * h:32 * h + 32, g, 128 * mt:128 * (mt + 1)],
                        q_sb[32 * h:32 * h + 32, g, CH * c:CH * (c + 1)],
                        start=True, stop=True, tile_position=(32 * h, 0))
                gate[0] = nc.scalar.activation(pt[:, :, mt, :], sc[:, :, :CH],
                                               AF.Exp, scale=SCALE)
                consume(per_mt)
            if pre_osum is not None:
                pre_osum()
            osum = psum.tile([128, 4, 512], F32, tag="ps", name="ps_osum")
            for kt in range(NKT):
                for h in range(4):
                    nc.tensor.matmul(
                        osum[32 * h:32 * h + 32, 0, :CH],
                        vt[:, kt, 128 * g + 32 * h:128 * g + 32 * h + 32],
                        pt[:, h, kt, :],
                        start=(kt == 0), stop=(kt == NKT - 1),
                        tile_position=(0, 32 * h))
                for h in range(4):
                    nc.tensor.matmul(
                        osum[32 * h:32 * h + 32, 1, :CH],
                        ones_sb[:, 0:32],
                        pt[:, h, kt, :],
                        start=(kt == 0), stop=(kt == NKT - 1),
                        tile_position=(0, 32 * h))
            r_sb = work.tile([128, CH], F32, tag="recip", name="r_sb")
            nc.vector.reciprocal(r_sb[:], osum[:, 1, :CH])
            o_tmp = work.tile([128, CH], DT, tag="otmp", name="o_tmp")
            nc.vector.tensor_tensor(o_tmp[:], osum[:, 0, :CH], r_sb[:], ALU.mult)
            nc.vector.tensor_scalar_add(proj_in[:, g, CH * c:CH * (c + 1)],
                                        o_tmp[:], bias[:, 4 + g:5 + g])

        def emit_ffn(c):
            for g in range(2):
                sl = slice(CH * c, CH * (c + 1))
                nc.gpsimd.tensor_tensor(
                    proj_in[:, g, sl], proj_in[:, g, sl],
                    pe_a[:, g].rearrange("p a b -> p (a b)")[:, sl], ALU.add)
            for mt in range(2):
                ps = psum.tile([128, 4, 512], F32, tag="ps", name="ps_proj")
                for kt in range(2):
                    nc.tensor.matmul(
                        ps[:, 0, :CH],
                        w_proj[:, kt, 128 * mt:128 * (mt + 1)],
                        proj_in[:, kt, CH * c:CH * (c + 1)],
                        start=(kt == 0), stop=(kt == 1))
                nc.vector.scalar_tensor_tensor(
                    x1[:, mt, CH * c:CH * (c + 1)], ps[:, 0, :CH],
                    bias[:, BIAS_PROJ + mt:BIAS_PROJ + mt + 1],
                    xloc[:, mt, CH * c:CH * (c + 1)], ALU.add, ALU.add)
                nc.gpsimd.tensor_copy(x1b[:, mt, CH * c:CH * (c + 1)],
                                      x1[:, mt, CH * c:CH * (c + 1)])
            for mt in range(4):
                ps = psum.tile([128, 4, 512], F32, tag="ps", name="ps_fc1")
                for kt in range(2):
                    nc.tensor.matmul(
                        ps[:, 0, :CH],
                        w_fc1[:, kt, 128 * mt:128 * (mt + 1)],
                        x1b[:, kt, CH * c:CH * (c + 1)],
                        start=(kt == 0), stop=(kt == 1))
                nc.scalar.activation(h_sb[:, mt, CH * c:CH * (c + 1)], ps[:, 0, :CH],
                                     AF.Silu, bias=bias[:, BIAS_FC1 + mt:BIAS_FC1 + mt + 1])
            for mt in range(2):
                ps = psum.tile([128, 4, 512], F32, tag="ps", name="ps_fc2")
                for kt in range(4):
                    nc.tensor.matmul(
                        ps[:, 0, :CH],
                        w_fc2[:, kt, 128 * mt:128 * (mt + 1)],
                        h_sb[:, kt, CH * c:CH * (c + 1)],
                        start=(kt == 0), stop=(kt == 3))
                nc.vector.scalar_tensor_tensor(
                    y_sb[:, mt, CH * c:CH * (c + 1)], ps[:, 0, :CH],
                    bias[:, BIAS_FC2 + mt:BIAS_FC2 + mt + 1],
                    x1[:, mt, CH * c:CH * (c + 1)], ALU.add, ALU.add)
                nc.sync.dma_start(ap_y[mt, :, CH * c:CH * (c + 1)],
                                  y_sb[:, mt, CH * c:CH * (c + 1)])

        # ---- main pipeline ----
        emit_attention(0, 0, per_mt=2)          # consumes convs/vT
        emit_attention(1, 0, per_mt=3)          # consumes taps
        emit_attention(0, 1, per_mt=3)
        emit_attention(1, 1, per_mt=2, pre_osum=lambda: (consume(len(queue)), emit_ffn(0)))
        emit_ffn(1)
    nc.compile()
    return nc


_CACHED = {}


def _get_nc():
    if "nc" not in _CACHED:
        _CACHED["nc"] = build_kernel()
    return _CACHED["nc"]


def _prep_inputs(inputs):
    x = np.asarray(inputs["x"], np.float32)           # [2, 256, 48, 48]
    qk_w = np.asarray(inputs["qk_w"], np.float32)
    qk_b = np.asarray(inputs["qk_b"], np.float32)
    v_w = np.asarray(inputs["v_w"], np.float32)
    v_b = np.asarray(inputs["v_b"], np.float32)
    pe_w = np.asarray(inputs["pe_w"], np.float32)
    pe_b = np.asarray(inputs["pe_b"], np.float32)
    proj_w = np.asarray(inputs["proj_w"], np.float32)
    proj_b = np.asarray(inputs["proj_b"], np.float32)
    fc1_w = np.asarray(inputs["fc1_w"], np.float32)
    fc1_b = np.asarray(inputs["fc1_b"], np.float32)
    fc2_w = np.asarray(inputs["fc2_w"], np.float32)
    fc2_b = np.asarray(inputs["fc2_b"], np.float32)

    rows = np.arange(2 * C).reshape(NH, 2, HD)
    q_rows = rows[:, 0, :].reshape(-1)
    k_rows = rows[:, 1, :].reshape(-1)
    w_qkv = np.concatenate([qk_w[q_rows], qk_w[k_rows], v_w], axis=0)  # [768, 256]
    wqkv = np.ascontiguousarray(w_qkv.T.reshape(2, 128, 768)).astype(BF16)
    wvt = np.ascontiguousarray(v_w.T.reshape(2, 128, 256)).astype(BF16)
    wproj = np.ascontiguousarray(proj_w.T.reshape(2, 128, 256)).astype(BF16)
    wfc1 = np.ascontiguousarray(fc1_w.T.reshape(2, 128, 512)).astype(BF16)
    wfc2 = np.ascontiguousarray(fc2_w.T.reshape(4, 128, 256)).astype(BF16)
    pew = np.ascontiguousarray(pe_w[:, 0].reshape(2, 128, 49)).astype(np.float32)

    bias = np.zeros((16, 128), np.float32)
    bias[0:2] = qk_b[q_rows].reshape(2, 128)
    bias[2:4] = qk_b[k_rows].reshape(2, 128)
    bias[4:6] = v_b.reshape(2, 128)
    bias[6:8] = proj_b.reshape(2, 128)
    bias[8:12] = fc1_b.reshape(4, 128)
    bias[12:14] = fc2_b.reshape(2, 128)
    bias[14:16] = pe_b.reshape(2, 128)

    xn = x.reshape(B, C, HS, WS)
    in_maps = []
    for core in range(N_CORES):
        b, r = core // 4, core % 4
        xrot = np.roll(xn[b], -12 * r, axis=1)                 # rotate rows
        xb = np.ascontiguousarray(xrot.reshape(C, N).reshape(2, 128, N)).astype(BF16)
        xloc = np.ascontiguousarray(
            xrot[:, :12, :].reshape(C, NQ).reshape(2, 128, NQ)).astype(np.float32)
        mask = np.ones((18, 54), np.float32)
        if r == 0:
            mask[0:3, :] = 0.0                                  # top image border
        if r == 3:
            mask[15:18, :] = 0.0                                # bottom image border
        pemask = np.broadcast_to(mask.reshape(1, 972), (128, 972)).astype(BF16)
        in_maps.append({
            "xb": xb, "xloc": xloc, "pemask": np.ascontiguousarray(pemask),
            "wqkv": wqkv, "wvt": wvt, "wproj": wproj, "wfc1": wfc1, "wfc2": wfc2,
            "pew": pew, "bias": bias,
        })
    return in_maps


def kernel(**inputs) -> np.ndarray:
    nc = _get_nc()
    in_maps = _prep_inputs(inputs)
    res = run_bass_kernel_spmd(nc, in_maps, core_ids=list(range(N_CORES)),
                               trace=False)
    out = np.zeros((B, C, HS, WS), np.float32)
    for core in range(N_CORES):
        b, r = core // 4, core % 4
        y = res.results[core]["y"].reshape(C, 12, WS)
        out[b, :, 12 * r:12 * (r + 1), :] = y
    return out


def run_traced(inputs):
    """test-harness helper: run with NTFF tracing, return (out, results)."""
    nc = _get_nc()
    in_maps = _prep_inputs(inputs)
    res = run_bass_kernel_spmd(nc, in_maps, core_ids=list(range(N_CORES)),
                               trace=True)
    out = np.zeros((B, C, HS, WS), np.float32)
    for core in range(N_CORES):
        b, r = core // 4, core % 4
        y = res.results[core]["y"].reshape(C, 12, WS)
        out[b, :, 12 * r:12 * (r + 1), :] = y
    return out, res


# revision 11
# speedup vs baseline: 1.1247x; 1.1079x over previous
"""Trainium2 Bass kernel for nn_AttentionBlockV2 (dense transformer block).

Sharding: 8 cores; core c handles batch b=c//4, image row-block r=c%4
(12 of 48 rows = 576 query pixels). Each core computes q/k/v for its FULL
batch (2304 keys; spatially rotated so the program is uniform across cores),
runs attention for its 576 queries over all keys, then the depthwise pos-enc
conv, projection, residuals and the conv-FFN for its local pixels.
No collectives: the host reassembles the 8 local outputs.

Self-contained: hardcodes all shapes; imports only numpy/ml_dtypes/concourse.
"""
import sys
import numpy as np
import ml_dtypes

try:
    import concourse.bass  # noqa: F401
except ImportError:  # fallback when the axon site path isn't preloaded
    sys.path.insert(0, "/opt/trn_rl_repo")

import bass_rust
import concourse.bass as bass
from concourse import bacc
import concourse.mybir as mybir
import concourse.tile as tile
from concourse.tile import add_dep_helper
from concourse.vector_clock import ScopedClock
from concourse.bass_utils import run_bass_kernel_spmd

BF16 = ml_dtypes.bfloat16
DT = mybir.dt.bfloat16
F32 = mybir.dt.float32
AF = mybir.ActivationFunctionType
ALU = mybir.AluOpType

# problem constants
B, C, NH, HD, KK, H1 = 2, 256, 8, 32, 7, 512
HS = WS = 48
N = HS * WS            # 2304 keys per batch
NQ = 576               # local queries per core
CH = 288               # query chunk (2 per core)
NKT = N // 128         # 18 key tiles
SCALE = HD ** -0.5
N_CORES = 8

# bias row layout in the packed [16, 128] bias tensor
BIAS_QKV = 0    # rows 0-5: q0,q1,k0,k1,v0,v1 (also v_b for o-norm at rows 4-5)
BIAS_PROJ = 6   # rows 6-7
BIAS_FC1 = 8    # rows 8-11
BIAS_FC2 = 12   # rows 12-13
BIAS_PE = 14    # rows 14-15


def _patched_drain_and_barrier(self, tick_clock, wait_clock):
    # upstream emits one epilogue drain carrying every outstanding wait;
    # walrus codegen accepts at most one sync wait per CTRL instruction,
    # so spread the extras over additional drains.
    drain_inst = self.nc.sync.drain()
    wait_clock.add_sem_waits(drain_inst.ins, ScopedClock({None: tick_clock.global_clock}))
    si = drain_inst.ins.sync_info
    waits = list(si.on_wait) if si is not None else []
    if len(waits) > 1:
        si.on_wait = [waits[0]]
        drain_inst.ins.sync_info = si
        for w in waits[1:]:
            extra = self.nc.sync.drain()
            extra.ins.sync_info = bass_rust.SyncInfo(on_wait=[w], on_update=[])
    self.nc.all_engine_barrier()
    assert self.sems is not None
    popped = self.nc._tile_sem_poison_stack.pop()
    assert popped is self._sem_poison
    self.nc.clear_and_free_semaphores(list(self.sems.allocated().values()))
    self.nc.all_engine_barrier()


tile.TileContext._drain_and_barrier = _patched_drain_and_barrier

# taps executed on the vector engine (rest go to gpsimd); tuned for balance
N_DVE_TAPS = 37


def build_kernel():
    from contextlib import ExitStack

    nc = bacc.Bacc("TRN2", target_bir_lowering=False, debug=False)
    ap_xb = nc.dram_tensor("xb", (2, 128, N), DT, kind="ExternalInput").ap()
    ap_xloc = nc.dram_tensor("xloc", (2, 128, NQ), F32, kind="ExternalInput").ap()
    ap_pemask = nc.dram_tensor("pemask", (128, 18 * 54), DT, kind="ExternalInput").ap()
    ap_wqkv = nc.dram_tensor("wqkv", (2, 128, 768), DT, kind="ExternalInput").ap()
    ap_wvt = nc.dram_tensor("wvt", (2, 128, 256), DT, kind="ExternalInput").ap()
    ap_wproj = nc.dram_tensor("wproj", (2, 128, 256), DT, kind="ExternalInput").ap()
    ap_wfc1 = nc.dram_tensor("wfc1", (2, 128, 512), DT, kind="ExternalInput").ap()
    ap_wfc2 = nc.dram_tensor("wfc2", (4, 128, 256), DT, kind="ExternalInput").ap()
    ap_pew = nc.dram_tensor("pew", (2, 128, 49), F32, kind="ExternalInput").ap()
    ap_bias = nc.dram_tensor("bias", (16, 128), F32, kind="ExternalInput").ap()
    ap_y = nc.dram_tensor("y", (2, 128, NQ), F32, kind="ExternalOutput").ap()

    with tile.TileContext(nc) as tc, ExitStack() as ctx:
        const = ctx.enter_context(tc.tile_pool(name="const", bufs=1))
        persist = ctx.enter_context(tc.tile_pool(name="persist", bufs=1))
        work = ctx.enter_context(tc.tile_pool(name="work", bufs=2))
        ppool = ctx.enter_context(tc.tile_pool(name="pbuf", bufs=2))
        psum = ctx.enter_context(tc.tile_pool(name="psum", bufs=2, space="PSUM"))

        # ---- loads ----
        w_qkv = const.tile([128, 2, 768], DT)
        nc.sync.dma_start(w_qkv[:], ap_wqkv.rearrange("a p m -> p a m"))
        w_vt = const.tile([128, 2, 256], DT)
        nc.sync.dma_start(w_vt[:], ap_wvt.rearrange("a p m -> p a m"))
        w_proj = const.tile([128, 2, 256], DT)
        nc.sync.dma_start(w_proj[:], ap_wproj.rearrange("a p m -> p a m"))
        w_fc1 = const.tile([128, 2, 512], DT)
        nc.sync.dma_start(w_fc1[:], ap_wfc1.rearrange("a p m -> p a m"))
        w_fc2 = const.tile([128, 4, 256], DT)
        nc.sync.dma_start(w_fc2[:], ap_wfc2.rearrange("a p m -> p a m"))
        pew = const.tile([128, 2, 49], F32)
        nc.sync.dma_start(pew[:], ap_pew.rearrange("a p m -> p a m"))
        bias = const.tile([128, 16], F32)
        nc.sync.dma_start(bias[:], ap_bias.rearrange("a p -> p a"))
        pemask = const.tile([128, 18, 54], DT)
        nc.sync.dma_start(pemask[:], ap_pemask.rearrange("p (a b) -> p a b", a=18))
        ones_sb = const.tile([128, 32], DT)
        nc.vector.memset(ones_sb[:], 1.0)

        NCH = [(0, 512), (512, 512), (1024, 512), (1536, 512), (2048, 256)]
        x_sb = persist.tile([128, 2, N], DT)
        for c0, cw in NCH:   # chunked so convs start before the full x lands
            nc.sync.dma_start(x_sb[:, :, c0:c0 + cw], ap_xb.rearrange("a p n -> p a n")[:, :, c0:c0 + cw])
        xloc = persist.tile([128, 2, NQ], F32)
        nc.sync.dma_start(xloc[:], ap_xloc.rearrange("a p n -> p a n"))

        # ---- prologue: k conv chunk0 + q conv (rest of k via work queue) ----
        k_sb = persist.tile([128, 2, N], DT)

        def emit_kconv(arg):
            c0, cw = arg
            for mt in range(2):
                ps = psum.tile([128, 4, 512], F32, tag="ps", name="ps_k")
                for kt in range(2):
                    nc.tensor.matmul(
                        ps[:, 0, :cw],
                        w_qkv[:, kt, 128 * (2 + mt):128 * (3 + mt)],
                        x_sb[:, kt, c0:c0 + cw],
                        start=(kt == 0), stop=(kt == 1))
                nc.scalar.activation(k_sb[:, mt, c0:c0 + cw], ps[:, 0, :cw],
                                     AF.Identity, bias=bias[:, 2 + mt:3 + mt])

        emit_kconv(NCH[0])
        q_sb = persist.tile([128, 2, NQ], DT)
        for mt in range(2):
            for c0, cw in [(0, 512), (512, 64)]:
                ps = psum.tile([128, 4, 512], F32, tag="ps", name="ps_q")
                for kt in range(2):
                    nc.tensor.matmul(
                        ps[:, 0, :cw],
                        w_qkv[:, kt, 128 * mt:128 * (mt + 1)],
                        x_sb[:, kt, c0:c0 + cw],
                        start=(kt == 0), stop=(kt == 1))
                nc.scalar.activation(q_sb[:, mt, c0:c0 + cw], ps[:, 0, :cw],
                                     AF.Identity, bias=bias[:, mt:mt + 1])

        # persistent tiles used by interleaved work
        vt = persist.tile([128, NKT, 256], DT)
        vpad = persist.tile([128, 2, 18, 54], DT)
        vpad1 = persist.tile([128, 2, 18, 54], DT)
        pe_a = persist.tile([128, 2, 12, 48], DT)
        pe_b = persist.tile([128, 2, 12, 48], DT)
        proj_in = persist.tile([128, 2, NQ], DT)
        x1 = persist.tile([128, 2, NQ], F32)
        x1b = persist.tile([128, 2, NQ], DT)
        h_sb = persist.tile([128, 4, NQ], DT)
        y_sb = persist.tile([128, 2, NQ], F32)
        nc.gpsimd.memset(vpad[:], 0.0)

        VW = [(2160, 144, 0, 3), (0, 480, 3, 10), (480, 240, 13, 5)]

        def emit_vconv(job):
            ct, (c0, cw, r0, nr) = job
            ps = psum.tile([128, 4, 512], F32, tag="ps", name="ps_v")
            for kt in range(2):
                nc.tensor.matmul(
                    ps[:, 0, :cw],
                    w_qkv[:, kt, 128 * (4 + ct):128 * (5 + ct)],
                    x_sb[:, kt, c0:c0 + cw],
                    start=(kt == 0), stop=(kt == 1))
            nc.vector.scalar_tensor_tensor(
                vpad[:, ct, r0:r0 + nr, 3:51],
                ps[:, 0, :cw].rearrange("p (a b) -> p a b", b=48),
                bias[:, 4 + ct:5 + ct],
                pemask[:, r0:r0 + nr, 3:51],
                ALU.add, ALU.mult)

        def emit_vpad1(_):
            nc.vector.tensor_copy(vpad1[:, :, :, 0:53], vpad[:, :, :, 1:54])

        def emit_vt(mt):
            ps = psum.tile([128, 4, 512], F32, tag="ps", name="ps_vt")
            for kt in range(2):
                nc.tensor.matmul(
                    ps[:, 0, :256],
                    x_sb[:, kt, 128 * mt:128 * (mt + 1)],
                    w_vt[:, kt, :],
                    start=(kt == 0), stop=(kt == 1))
            nc.vector.tensor_copy(vt[:, mt, :], ps[:, 0, :256])

        pe_b_started = [False, False]
        gate = [None]

        def _gated(bi):
            if gate[0] is not None:
                add_dep_helper(bi.ins, gate[0].ins, sync=False,
                               reason="pace filler work to the exp cadence")
            return bi

        def emit_tap(job):
            kind, ct, dy, dx, first = job
            if dx % 2 == 0:
                win = vpad[:, ct, dy:dy + 12, dx:dx + 48]
            else:
                win = vpad1[:, ct, dy:dy + 12, dx - 1:dx + 47]
            sc = pew[:, ct, dy * 7 + dx:dy * 7 + dx + 1]
            if kind == "dve":
                if first:
                    _gated(nc.vector.tensor_scalar(pe_a[:, ct], win, sc,
                                                   bias[:, BIAS_PE + ct:BIAS_PE + ct + 1],
                                                   ALU.mult, ALU.add))
                else:
                    _gated(nc.vector.scalar_tensor_tensor(pe_a[:, ct], win, sc,
                                                          pe_a[:, ct], ALU.mult, ALU.add))
            else:
                tmp = work.tile([128, 12, 48], DT, tag="ttmp", name="ttmp")
                _gated(nc.vector.tensor_scalar_mul(tmp[:], win, sc))
                if not pe_b_started[ct]:
                    pe_b_started[ct] = True
                    _gated(nc.gpsimd.tensor_copy(pe_b[:, ct], tmp[:]))
                else:
                    _gated(nc.gpsimd.tensor_tensor(pe_b[:, ct], pe_b[:, ct], tmp[:], ALU.add))

        def emit_pe_merge(ct):
            _gated(nc.gpsimd.tensor_tensor(pe_a[:, ct], pe_a[:, ct], pe_b[:, ct], ALU.add))

        # build the paced work queue
        taps = [(dy, dx) for dy in range(7) for dx in range(7)]
        queue = []
        for ch in NCH[1:]:
            queue.append((emit_kconv, ch))
        for ct in range(2):
            for vw in VW:
                queue.append((emit_vconv, (ct, vw)))
        queue.append((emit_vpad1, None))
        for mt in range(NKT):
            queue.append((emit_vt, mt))
        phase1_len = len(queue)           # consumed during att(0,0)
        for ct in range(2):
            dve_jobs = [("dve", ct, dy, dx, i == 0)
                        for i, (dy, dx) in enumerate(taps[:29])]
            gps_jobs = [("gps", ct, dy, dx, False) for dy, dx in taps[29:]]
            mixed = []
            di, gi = 0, 0
            for j in range(49):
                # ~3 dve per 2 gps keeps both engines evenly fed
                if (j * 20) // 49 > gi - 1 and gi < 20 and (j % 5 >= 3 or di >= 29):
                    mixed.append(gps_jobs[gi]); gi += 1
                elif di < 29:
                    mixed.append(dve_jobs[di]); di += 1
                else:
                    mixed.append(gps_jobs[gi]); gi += 1
            for job in mixed:
                queue.append((emit_tap, job))
        for ct in range(2):
            queue.append((emit_pe_merge, ct))
        qpos = [0]

        def consume(n):
            lim = min(qpos[0] + n, len(queue))
            while qpos[0] < lim:
                fn, arg = queue[qpos[0]]
                qpos[0] += 1
                fn(arg)

        def emit_attention(g, c, per_mt, pre_osum=None):
            pt = ppool.tile([128, 4, NKT, CH], DT, tag="P", name=f"P_{g}_{c}")
            for mt in range(NKT):
                sc = psum.tile([128, 4, 512], F32, tag="ps", name="ps_sc")
                for h in range(4):
                    nc.tensor.matmul(
                        sc[:, h, :CH],
                        k_sb[32 * h:32 * h + 32, g, 128 * mt:128 * (mt + 1)],
                        q_sb[32 * h:32 * h + 32, g, CH * c:CH * (c + 1)],
                        start=True, stop=True, tile_position=(32 * h, 0))
                gate[0] = nc.scalar.activation(pt[:, :, mt, :], sc[:, :, :CH],
                                               AF.Exp, scale=SCALE)
                consume(per_mt)
            if pre_osum is not None:
                pre_osum()
            osum = psum.tile([128, 4, 512], F32, tag="ps", name="ps_osum")
            for kt in range(NKT):
                for h in range(4):
                    nc.tensor.matmul(
                        osum[32 * h:32 * h + 32, 0, :CH],
                        vt[:, kt, 128 * g + 32 * h:128 * g + 32 * h + 32],
                        pt[:, h, kt, :],
                        start=(kt == 0), stop=(kt == NKT - 1),
                        tile_position=(0, 32 * h))
                for h in range(4):
                    nc.tensor.matmul(
                        osum[32 * h:32 * h + 32, 1, :CH],
                        ones_sb[:, 0:32],
                        pt[:, h, kt, :],
                        start=(kt == 0), stop=(kt == NKT - 1),
                        tile_position=(0, 32 * h))
            r_sb = work.tile([128, CH], F32, tag="recip", name="r_sb")
            nc.vector.reciprocal(r_sb[:], osum[:, 1, :CH])
            o_tmp = work.tile([128, CH], DT, tag="otmp", name="o_tmp")
            nc.vector.tensor_tensor(o_tmp[:], osum[:, 0, :CH], r_sb[:], ALU.mult)
            nc.vector.tensor_scalar_add(proj_in[:, g, CH * c:CH * (c + 1)],
                                        o_tmp[:], bias[:, 4 + g:5 + g])

        def emit_ffn(c):
            for g in range(2):
                sl = slice(CH * c, CH * (c + 1))
                nc.gpsimd.tensor_tensor(
                    proj_in[:, g, sl], proj_in[:, g, sl],
                    pe_a[:, g].rearrange("p a b -> p (a b)")[:, sl], ALU.add)
            for mt in range(2):
                ps = psum.tile([128, 4, 512], F32, tag="ps", name="ps_proj")
                for kt in range(2):
                    nc.tensor.matmul(
                        ps[:, 0, :CH],
                        w_proj[:, kt, 128 * mt:128 * (mt + 1)],
                        proj_in[:, kt, CH * c:CH * (c + 1)],
                        start=(kt == 0), stop=(kt == 1))
                nc.vector.scalar_tensor_tensor(
                    x1[:, mt, CH * c:CH * (c + 1)], ps[:, 0, :CH],
                    bias[:, BIAS_PROJ + mt:BIAS_PROJ + mt + 1],
                    xloc[:, mt, CH * c:CH * (c + 1)], ALU.add, ALU.add)
                nc.gpsimd.tensor_copy(x1b[:, mt, CH * c:CH * (c + 1)],
                                      x1[:, mt, CH * c:CH * (c + 1)])
            for mt in range(4):
                ps = psum.tile([128, 4, 512], F32, tag="ps", name="ps_fc1")
                for kt in range(2):
                    nc.tensor.matmul(
                        ps[:, 0, :CH],
                        w_fc1[:, kt, 128 * mt:128 * (mt + 1)],
                        x1b[:, kt, CH * c:CH * (c + 1)],
                        start=(kt == 0), stop=(kt == 1))
                nc.scalar.activation(h_sb[:, mt, CH * c:CH * (c + 1)], ps[:, 0, :CH],
                                     AF.Silu, bias=bias[:, BIAS_FC1 + mt:BIAS_FC1 + mt + 1])
            for mt in range(2):
                ps = psum.tile([128, 4, 512], F32, tag="ps", name="ps_fc2")
                for kt in range(4):
                    nc.tensor.matmul(
                        ps[:, 0, :CH],
                        w_fc2[:, kt, 128 * mt:128 * (mt + 1)],
                        h_sb[:, kt, CH * c:CH * (c + 1)],
                        start=(kt == 0), stop=(kt == 3))
                nc.vector.scalar_tensor_tensor(
                    y_sb[:, mt, CH * c:CH * (c + 1)], ps[:, 0, :CH],
                    bias[:, BIAS_FC2 + mt:BIAS_FC2 + mt + 1],
                    x1[:, mt, CH * c:CH * (c + 1)], ALU.add, ALU.add)
                nc.sync.dma_start(ap_y[mt, :, CH * c:CH * (c + 1)],
                                  y_sb[:, mt, CH * c:CH * (c + 1)])

        # ---- main pipeline ----
        emit_attention(0, 0, per_mt=2)          # consumes convs/vT
        emit_attention(1, 0, per_mt=2)          # consumes taps
        emit_attention(0, 1, per_mt=2)
        emit_attention(1, 1, per_mt=3, pre_osum=lambda: (consume(len(queue)), emit_ffn(0)))
        emit_ffn(1)
    nc.compile()
    return nc


_CACHED = {}


def _get_nc():
    if "nc" not in _CACHED:
        _CACHED["nc"] = build_kernel()
    return _CACHED["nc"]


def _prep_inputs(inputs):
    x = np.asarray(inputs["x"], np.float32)           # [2, 256, 48, 48]
    qk_w = np.asarray(inputs["qk_w"], np.float32)
    qk_b = np.asarray(inputs["qk_b"], np.float32)
    v_w = np.asarray(inputs["v_w"], np.float32)
    v_b = np.asarray(inputs["v_b"], np.float32)
    pe_w = np.asarray(inputs["pe_w"], np.float32)
    pe_b = np.asarray(inputs["pe_b"], np.float32)
    proj_w = np.asarray(inputs["proj_w"], np.float32)
    proj_b = np.asarray(inputs["proj_b"], np.float32)
    fc1_w = np.asarray(inputs["fc1_w"], np.float32)
    fc1_b = np.asarray(inputs["fc1_b"], np.float32)
    fc2_w = np.asarray(inputs["fc2_w"], np.float32)
    fc2_b = np.asarray(inputs["fc2_b"], np.float32)

    rows = np.arange(2 * C).reshape(NH, 2, HD)
    q_rows = rows[:, 0, :].reshape(-1)
    k_rows = rows[:, 1, :].reshape(-1)
    w_qkv = np.concatenate([qk_w[q_rows], qk_w[k_rows], v_w], axis=0)  # [768, 256]
    wqkv = np.ascontiguousarray(w_qkv.T.reshape(2, 128, 768)).astype(BF16)
    wvt = np.ascontiguousarray(v_w.T.reshape(2, 128, 256)).astype(BF16)
    wproj = np.ascontiguousarray(proj_w.T.reshape(2, 128, 256)).astype(BF16)
    wfc1 = np.ascontiguousarray(fc1_w.T.reshape(2, 128, 512)).astype(BF16)
    wfc2 = np.ascontiguousarray(fc2_w.T.reshape(4, 128, 256)).astype(BF16)
    pew = np.ascontiguousarray(pe_w[:, 0].reshape(2, 128, 49)).astype(np.float32)

    bias = np.zeros((16, 128), np.float32)
    bias[0:2] = qk_b[q_rows].reshape(2, 128)
    bias[2:4] = qk_b[k_rows].reshape(2, 128)
    bias[4:6] = v_b.reshape(2, 128)
    bias[6:8] = proj_b.reshape(2, 128)
    bias[8:12] = fc1_b.reshape(4, 128)
    bias[12:14] = fc2_b.reshape(2, 128)
    bias[14:16] = pe_b.reshape(2, 128)

    xn = x.reshape(B, C, HS, WS)
    in_maps = []
    for core in range(N_CORES):
        b, r = core // 4, core % 4
        xrot = np.roll(xn[b], -12 * r, axis=1)                 # rotate rows
        xb = np.ascontiguousarray(xrot.reshape(C, N).reshape(2, 128, N)).astype(BF16)
        xloc = np.ascontiguousarray(
            xrot[:, :12, :].reshape(C, NQ).reshape(2, 128, NQ)).astype(np.float32)
        mask = np.ones((18, 54), np.float32)
        if r == 0:
            mask[0:3, :] = 0.0                                  # top image border
        if r == 3:
            mask[15:18, :] = 0.0                                # bottom image border
        pemask = np.broadcast_to(mask.reshape(1, 972), (128, 972)).astype(BF16)
        in_maps.append({
            "xb": xb, "xloc": xloc, "pemask": np.ascontiguousarray(pemask),
            "wqkv": wqkv, "wvt": wvt, "wproj": wproj, "wfc1": wfc1, "wfc2": wfc2,
            "pew": pew, "bias": bias,
        })
    return in_maps


def kernel(**inputs) -> np.ndarray:
    nc = _get_nc()
    in_maps = _prep_inputs(inputs)
    res = run_bass_kernel_spmd(nc, in_maps, core_ids=list(range(N_CORES)),
                               trace=False)
    out = np.zeros((B, C, HS, WS), np.float32)
    for core in range(N_CORES):
        b, r = core // 4, core % 4
        y = res.results[core]["y"].reshape(C, 12, WS)
        out[b, :, 12 * r:12 * (r + 1), :] = y
    return out


def run_traced(inputs):
    """test-harness helper: run with NTFF tracing, return (out, results)."""
    nc = _get_nc()
    in_maps = _prep_inputs(inputs)
    res = run_bass_kernel_spmd(nc, in_maps, core_ids=list(range(N_CORES)),
                               trace=True)
    out = np.zeros((B, C, HS, WS), np.float32)
    for core in range(N_CORES):
        b, r = core // 4, core % 4
        y = res.results[core]["y"].reshape(C, 12, WS)
        out[b, :, 12 * r:12 * (r + 1), :] = y
    return out, res


# revision 12
# speedup vs baseline: 1.1826x; 1.0514x over previous
"""Trainium2 Bass kernel for nn_AttentionBlockV2 (dense transformer block).

Sharding: 8 cores; core c handles batch b=c//4, image row-block r=c%4
(12 of 48 rows = 576 query pixels). Each core computes q/k/v for its FULL
batch (2304 keys; spatially rotated so the program is uniform across cores),
runs attention for its 576 queries over all keys, then the depthwise pos-enc
conv, projection, residuals and the conv-FFN for its local pixels.
No collectives: the host reassembles the 8 local outputs.

Self-contained: hardcodes all shapes; imports only numpy/ml_dtypes/concourse.
"""
import sys
import numpy as np
import ml_dtypes

try:
    import concourse.bass  # noqa: F401
except ImportError:  # fallback when the axon site path isn't preloaded
    sys.path.insert(0, "/opt/trn_rl_repo")

import bass_rust
import concourse.bass as bass
from concourse import bacc
import concourse.mybir as mybir
import concourse.tile as tile
from concourse.tile import add_dep_helper
from concourse.vector_clock import ScopedClock
from concourse.bass_utils import run_bass_kernel_spmd

BF16 = ml_dtypes.bfloat16
DT = mybir.dt.bfloat16
F32 = mybir.dt.float32
AF = mybir.ActivationFunctionType
ALU = mybir.AluOpType

# problem constants
B, C, NH, HD, KK, H1 = 2, 256, 8, 32, 7, 512
HS = WS = 48
N = HS * WS            # 2304 keys per batch
NQ = 576               # local queries per core
CH = 288               # query chunk (2 per core)
NKT = N // 128         # 18 key tiles
SCALE = HD ** -0.5
N_CORES = 8

# bias row layout in the packed [16, 128] bias tensor
BIAS_QKV = 0    # rows 0-5: q0,q1,k0,k1,v0,v1 (also v_b for o-norm at rows 4-5)
BIAS_PROJ = 6   # rows 6-7
BIAS_FC1 = 8    # rows 8-11
BIAS_FC2 = 12   # rows 12-13
BIAS_PE = 14    # rows 14-15


def _patched_drain_and_barrier(self, tick_clock, wait_clock):
    # upstream emits one epilogue drain carrying every outstanding wait;
    # walrus codegen accepts at most one sync wait per CTRL instruction,
    # so spread the extras over additional drains.
    drain_inst = self.nc.sync.drain()
    wait_clock.add_sem_waits(drain_inst.ins, ScopedClock({None: tick_clock.global_clock}))
    si = drain_inst.ins.sync_info
    waits = list(si.on_wait) if si is not None else []
    if len(waits) > 1:
        si.on_wait = [waits[0]]
        drain_inst.ins.sync_info = si
        for w in waits[1:]:
            extra = self.nc.sync.drain()
            extra.ins.sync_info = bass_rust.SyncInfo(on_wait=[w], on_update=[])
    self.nc.all_engine_barrier()
    assert self.sems is not None
    popped = self.nc._tile_sem_poison_stack.pop()
    assert popped is self._sem_poison
    self.nc.clear_and_free_semaphores(list(self.sems.allocated().values()))
    self.nc.all_engine_barrier()


tile.TileContext._drain_and_barrier = _patched_drain_and_barrier

# taps executed on the vector engine (rest go to gpsimd); tuned for balance
N_DVE_TAPS = 37


def build_kernel():
    from contextlib import ExitStack

    nc = bacc.Bacc("TRN2", target_bir_lowering=False, debug=False)
    ap_xb = nc.dram_tensor("xb", (2, 128, N), DT, kind="ExternalInput").ap()
    ap_xloc = nc.dram_tensor("xloc", (2, 128, NQ), F32, kind="ExternalInput").ap()
    ap_pemask = nc.dram_tensor("pemask", (128, 18 * 54), DT, kind="ExternalInput").ap()
    ap_wqkv = nc.dram_tensor("wqkv", (2, 128, 768), DT, kind="ExternalInput").ap()
    ap_wvt = nc.dram_tensor("wvt", (2, 128, 256), DT, kind="ExternalInput").ap()
    ap_wproj = nc.dram_tensor("wproj", (2, 128, 256), DT, kind="ExternalInput").ap()
    ap_wfc1 = nc.dram_tensor("wfc1", (2, 128, 512), DT, kind="ExternalInput").ap()
    ap_wfc2 = nc.dram_tensor("wfc2", (4, 128, 256), DT, kind="ExternalInput").ap()
    ap_pew = nc.dram_tensor("pew", (2, 128, 49), F32, kind="ExternalInput").ap()
    ap_bias = nc.dram_tensor("bias", (16, 128), F32, kind="ExternalInput").ap()
    ap_y = nc.dram_tensor("y", (2, 128, NQ), F32, kind="ExternalOutput").ap()

    with tile.TileContext(nc) as tc, ExitStack() as ctx:
        const = ctx.enter_context(tc.tile_pool(name="const", bufs=1))
        persist = ctx.enter_context(tc.tile_pool(name="persist", bufs=1))
        work = ctx.enter_context(tc.tile_pool(name="work", bufs=2))
        ppool = ctx.enter_context(tc.tile_pool(name="pbuf", bufs=2))
        psum = ctx.enter_context(tc.tile_pool(name="psum", bufs=2, space="PSUM"))

        # ---- loads ----
        w_qkv = const.tile([128, 2, 768], DT)
        nc.sync.dma_start(w_qkv[:], ap_wqkv.rearrange("a p m -> p a m"))
        w_vt = const.tile([128, 2, 256], DT)
        nc.sync.dma_start(w_vt[:], ap_wvt.rearrange("a p m -> p a m"))
        w_proj = const.tile([128, 2, 256], DT)
        nc.sync.dma_start(w_proj[:], ap_wproj.rearrange("a p m -> p a m"))
        w_fc1 = const.tile([128, 2, 512], DT)
        nc.sync.dma_start(w_fc1[:], ap_wfc1.rearrange("a p m -> p a m"))
        w_fc2 = const.tile([128, 4, 256], DT)
        nc.sync.dma_start(w_fc2[:], ap_wfc2.rearrange("a p m -> p a m"))
        pew = const.tile([128, 2, 49], F32)
        nc.sync.dma_start(pew[:], ap_pew.rearrange("a p m -> p a m"))
        bias = const.tile([128, 16], F32)
        nc.sync.dma_start(bias[:], ap_bias.rearrange("a p -> p a"))
        pemask = const.tile([128, 18, 54], DT)
        nc.sync.dma_start(pemask[:], ap_pemask.rearrange("p (a b) -> p a b", a=18))
        ones_sb = const.tile([128, 32], DT)
        nc.vector.memset(ones_sb[:], 1.0)

        NCH = [(0, 512), (512, 512), (1024, 512), (1536, 512), (2048, 256)]
        x_sb = persist.tile([128, 2, N], DT)
        for c0, cw in NCH:   # chunked so convs start before the full x lands
            nc.sync.dma_start(x_sb[:, :, c0:c0 + cw], ap_xb.rearrange("a p n -> p a n")[:, :, c0:c0 + cw])
        xloc = persist.tile([128, 2, NQ], F32)
        nc.sync.dma_start(xloc[:], ap_xloc.rearrange("a p n -> p a n"))

        # ---- prologue: k conv chunk0 + q conv (rest of k via work queue) ----
        k_sb = persist.tile([128, 2, N], DT)

        def emit_kconv(arg):
            c0, cw = arg
            for mt in range(2):
                ps = psum.tile([128, 4, 512], F32, tag="ps", name="ps_k")
                for kt in range(2):
                    nc.tensor.matmul(
                        ps[:, 0, :cw],
                        w_qkv[:, kt, 128 * (2 + mt):128 * (3 + mt)],
                        x_sb[:, kt, c0:c0 + cw],
                        start=(kt == 0), stop=(kt == 1))
                nc.scalar.activation(k_sb[:, mt, c0:c0 + cw], ps[:, 0, :cw],
                                     AF.Identity, bias=bias[:, 2 + mt:3 + mt])

        emit_kconv(NCH[0])
        q_sb = persist.tile([128, 2, NQ], DT)
        for mt in range(2):
            for c0, cw in [(0, 512), (512, 64)]:
                ps = psum.tile([128, 4, 512], F32, tag="ps", name="ps_q")
                for kt in range(2):
                    nc.tensor.matmul(
                        ps[:, 0, :cw],
                        w_qkv[:, kt, 128 * mt:128 * (mt + 1)],
                        x_sb[:, kt, c0:c0 + cw],
                        start=(kt == 0), stop=(kt == 1))
                nc.scalar.activation(q_sb[:, mt, c0:c0 + cw], ps[:, 0, :cw],
                                     AF.Identity, bias=bias[:, mt:mt + 1])

        # persistent tiles used by interleaved work
        vt = persist.tile([128, NKT, 256], DT)
        vpad = persist.tile([128, 2, 18, 54], DT)
        vpad1 = persist.tile([128, 2, 18, 54], DT)
        pe_a = persist.tile([128, 2, 12, 48], DT)
        pe_b = persist.tile([128, 2, 12, 48], DT)
        proj_in = persist.tile([128, 2, NQ], DT)
        x1 = persist.tile([128, 2, NQ], F32)
        x1b = persist.tile([128, 2, NQ], DT)
        h_sb = persist.tile([128, 4, NQ], DT)
        y_sb = persist.tile([128, 2, NQ], F32)
        nc.gpsimd.memset(vpad[:], 0.0)

        VW = [(2160, 144, 0, 3), (0, 480, 3, 10), (480, 240, 13, 5)]

        def emit_vconv(job):
            ct, (c0, cw, r0, nr) = job
            ps = psum.tile([128, 4, 512], F32, tag="ps", name="ps_v")
            for kt in range(2):
                nc.tensor.matmul(
                    ps[:, 0, :cw],
                    w_qkv[:, kt, 128 * (4 + ct):128 * (5 + ct)],
                    x_sb[:, kt, c0:c0 + cw],
                    start=(kt == 0), stop=(kt == 1))
            nc.vector.scalar_tensor_tensor(
                vpad[:, ct, r0:r0 + nr, 3:51],
                ps[:, 0, :cw].rearrange("p (a b) -> p a b", b=48),
                bias[:, 4 + ct:5 + ct],
                pemask[:, r0:r0 + nr, 3:51],
                ALU.add, ALU.mult)

        def emit_vpad1(_):
            nc.vector.tensor_copy(vpad1[:, :, :, 0:53], vpad[:, :, :, 1:54])

        def emit_vt(mt):
            ps = psum.tile([128, 4, 512], F32, tag="ps", name="ps_vt")
            for kt in range(2):
                nc.tensor.matmul(
                    ps[:, 0, :256],
                    x_sb[:, kt, 128 * mt:128 * (mt + 1)],
                    w_vt[:, kt, :],
                    start=(kt == 0), stop=(kt == 1))
            nc.vector.tensor_copy(vt[:, mt, :], ps[:, 0, :256])

        pe_b_started = [False, False]
        gate = [None]

        def _gated(bi):
            if gate[0] is not None:
                add_dep_helper(bi.ins, gate[0].ins, sync=False,
                               reason="pace filler work to the exp cadence")
            return bi

        def emit_tap(job):
            kind, ct, dy, dx, first = job
            if dx % 2 == 0:
                win = vpad[:, ct, dy:dy + 12, dx:dx + 48]
            else:
                win = vpad1[:, ct, dy:dy + 12, dx - 1:dx + 47]
            sc = pew[:, ct, dy * 7 + dx:dy * 7 + dx + 1]
            if kind == "dve":
                if first:
                    _gated(nc.vector.tensor_scalar(pe_a[:, ct], win, sc,
                                                   bias[:, BIAS_PE + ct:BIAS_PE + ct + 1],
                                                   ALU.mult, ALU.add))
                else:
                    _gated(nc.vector.scalar_tensor_tensor(pe_a[:, ct], win, sc,
                                                          pe_a[:, ct], ALU.mult, ALU.add))
            else:
                tmp = work.tile([128, 12, 48], DT, tag="ttmp", name="ttmp")
                _gated(nc.vector.tensor_scalar_mul(tmp[:], win, sc))
                if not pe_b_started[ct]:
                    pe_b_started[ct] = True
                    _gated(nc.gpsimd.tensor_copy(pe_b[:, ct], tmp[:]))
                else:
                    _gated(nc.gpsimd.tensor_tensor(pe_b[:, ct], pe_b[:, ct], tmp[:], ALU.add))

        def emit_pe_merge(ct):
            _gated(nc.gpsimd.tensor_tensor(pe_a[:, ct], pe_a[:, ct], pe_b[:, ct], ALU.add))

        # build the paced work queue
        taps = [(dy, dx) for dy in range(7) for dx in range(7)]
        queue = []
        for ch in NCH[1:]:
            queue.append((emit_kconv, ch))
        for ct in range(2):
            for vw in VW:
                queue.append((emit_vconv, (ct, vw)))
        queue.append((emit_vpad1, None))
        for mt in range(NKT):
            queue.append((emit_vt, mt))
        phase1_len = len(queue)           # consumed during att(0,0)
        for ct in range(2):
            dve_jobs = [("dve", ct, dy, dx, i == 0)
                        for i, (dy, dx) in enumerate(taps[:43])]
            gps_jobs = [("gps", ct, dy, dx, False) for dy, dx in taps[43:]]
            mixed = []
            for j in range(49):
                if j % 8 == 7 and gps_jobs:
                    mixed.append(gps_jobs.pop(0))
                elif dve_jobs:
                    mixed.append(dve_jobs.pop(0))
                else:
                    mixed.append(gps_jobs.pop(0))
            for job in mixed:
                queue.append((emit_tap, job))
        for ct in range(2):
            queue.append((emit_pe_merge, ct))
        qpos = [0]

        def consume(n):
            lim = min(qpos[0] + n, len(queue))
            while qpos[0] < lim:
                fn, arg = queue[qpos[0]]
                qpos[0] += 1
                fn(arg)

        def emit_attention(g, c, per_mt, pre_osum=None):
            pt = ppool.tile([128, 4, NKT, CH], DT, tag="P", name=f"P_{g}_{c}")
            for mt in range(NKT):
                sc = psum.tile([128, 4, 512], F32, tag="ps", name="ps_sc")
                for h in range(4):
                    nc.tensor.matmul(
                        sc[:, h, :CH],
                        k_sb[32 * h:32 * h + 32, g, 128 * mt:128 * (mt + 1)],
                        q_sb[32 * h:32 * h + 32, g, CH * c:CH * (c + 1)],
                        start=True, stop=True, tile_position=(32 * h, 0))
                gate[0] = nc.scalar.activation(pt[:, :, mt, :], sc[:, :, :CH],
                                               AF.Exp, scale=SCALE)
                consume(per_mt)
            if pre_osum is not None:
                pre_osum()
            osum = psum.tile([128, 4, 512], F32, tag="ps", name="ps_osum")
            for kt in range(NKT):
                for h in range(4):
                    nc.tensor.matmul(
                        osum[32 * h:32 * h + 32, 0, :CH],
                        vt[:, kt, 128 * g + 32 * h:128 * g + 32 * h + 32],
                        pt[:, h, kt, :],
                        start=(kt == 0), stop=(kt == NKT - 1),
                        tile_position=(0, 32 * h))
                for h in range(4):
                    nc.tensor.matmul(
                        osum[32 * h:32 * h + 32, 1, :CH],
                        ones_sb[:, 0:32],
                        pt[:, h, kt, :],
                        start=(kt == 0), stop=(kt == NKT - 1),
                        tile_position=(0, 32 * h))
            r_sb = work.tile([128, CH], F32, tag="recip", name="r_sb")
            nc.vector.reciprocal(r_sb[:], osum[:, 1, :CH])
            o_tmp = work.tile([128, CH], DT, tag="otmp", name="o_tmp")
            nc.vector.tensor_tensor(o_tmp[:], osum[:, 0, :CH], r_sb[:], ALU.mult)
            nc.vector.tensor_scalar_add(proj_in[:, g, CH * c:CH * (c + 1)],
                                        o_tmp[:], bias[:, 4 + g:5 + g])

        def emit_ffn(c):
            for g in range(2):
                sl = slice(CH * c, CH * (c + 1))
                nc.vector.tensor_tensor(
                    proj_in[:, g, sl], proj_in[:, g, sl],
                    pe_a[:, g].rearrange("p a b -> p (a b)")[:, sl], ALU.add)
            for mt in range(2):
                ps = psum.tile([128, 4, 512], F32, tag="ps", name="ps_proj")
                for kt in range(2):
                    nc.tensor.matmul(
                        ps[:, 0, :CH],
                        w_proj[:, kt, 128 * mt:128 * (mt + 1)],
                        proj_in[:, kt, CH * c:CH * (c + 1)],
                        start=(kt == 0), stop=(kt == 1))
                nc.vector.scalar_tensor_tensor(
                    x1[:, mt, CH * c:CH * (c + 1)], ps[:, 0, :CH],
                    bias[:, BIAS_PROJ + mt:BIAS_PROJ + mt + 1],
                    xloc[:, mt, CH * c:CH * (c + 1)], ALU.add, ALU.add)
                nc.gpsimd.tensor_copy(x1b[:, mt, CH * c:CH * (c + 1)],
                                      x1[:, mt, CH * c:CH * (c + 1)])
            for mt in range(4):
                ps = psum.tile([128, 4, 512], F32, tag="ps", name="ps_fc1")
                for kt in range(2):
                    nc.tensor.matmul(
                        ps[:, 0, :CH],
                        w_fc1[:, kt, 128 * mt:128 * (mt + 1)],
                        x1b[:, kt, CH * c:CH * (c + 1)],
                        start=(kt == 0), stop=(kt == 1))
                nc.scalar.activation(h_sb[:, mt, CH * c:CH * (c + 1)], ps[:, 0, :CH],
                                     AF.Silu, bias=bias[:, BIAS_FC1 + mt:BIAS_FC1 + mt + 1])
            for mt in range(2):
                ps = psum.tile([128, 4, 512], F32, tag="ps", name="ps_fc2")
                for kt in range(4):
                    nc.tensor.matmul(
                        ps[:, 0, :CH],
                        w_fc2[:, kt, 128 * mt:128 * (mt + 1)],
                        h_sb[:, kt, CH * c:CH * (c + 1)],
                        start=(kt == 0), stop=(kt == 3))
                nc.vector.scalar_tensor_tensor(
                    y_sb[:, mt, CH * c:CH * (c + 1)], ps[:, 0, :CH],
                    bias[:, BIAS_FC2 + mt:BIAS_FC2 + mt + 1],
                    x1[:, mt, CH * c:CH * (c + 1)], ALU.add, ALU.add)
                nc.sync.dma_start(ap_y[mt, :, CH * c:CH * (c + 1)],
                                  y_sb[:, mt, CH * c:CH * (c + 1)])

        # ---- main pipeline ----
        emit_attention(0, 0, per_mt=2)          # consumes convs/vT
        emit_attention(1, 0, per_mt=2)          # consumes taps
        emit_attention(0, 1, per_mt=2)
        emit_attention(1, 1, per_mt=3, pre_osum=lambda: (consume(len(queue)), emit_ffn(0)))
        emit_ffn(1)
    nc.compile()
    return nc


_CACHED = {}


def _get_nc():
    if "nc" not in _CACHED:
        _CACHED["nc"] = build_kernel()
    return _CACHED["nc"]


def _prep_inputs(inputs):
    x = np.asarray(inputs["x"], np.float32)           # [2, 256, 48, 48]
    qk_w = np.asarray(inputs["qk_w"], np.float32)
    qk_b = np.asarray(inputs["qk_b"], np.float32)
    v_w = np.asarray(inputs["v_w"], np.float32)
    v_b = np.asarray(inputs["v_b"], np.float32)
    pe_w = np.asarray(inputs["pe_w"], np.float32)
    pe_b = np.asarray(inputs["pe_b"], np.float32)
    proj_w = np.asarray(inputs["proj_w"], np.float32)
    proj_b = np.asarray(inputs["proj_b"], np.float32)
    fc1_w = np.asarray(inputs["fc1_w"], np.float32)
    fc1_b = np.asarray(inputs["fc1_b"], np.float32)
    fc2_w = np.asarray(inputs["fc2_w"], np.float32)
    fc2_b = np.asarray(inputs["fc2_b"], np.float32)

    rows = np.arange(2 * C).reshape(NH, 2, HD)
    q_rows = rows[:, 0, :].reshape(-1)
    k_rows = rows[:, 1, :].reshape(-1)
    w_qkv = np.concatenate([qk_w[q_rows], qk_w[k_rows], v_w], axis=0)  # [768, 256]
    wqkv = np.ascontiguousarray(w_qkv.T.reshape(2, 128, 768)).astype(BF16)
    wvt = np.ascontiguousarray(v_w.T.reshape(2, 128, 256)).astype(BF16)
    wproj = np.ascontiguousarray(proj_w.T.reshape(2, 128, 256)).astype(BF16)
    wfc1 = np.ascontiguousarray(fc1_w.T.reshape(2, 128, 512)).astype(BF16)
    wfc2 = np.ascontiguousarray(fc2_w.T.reshape(4, 128, 256)).astype(BF16)
    pew = np.ascontiguousarray(pe_w[:, 0].reshape(2, 128, 49)).astype(np.float32)

    bias = np.zeros((16, 128), np.float32)
    bias[0:2] = qk_b[q_rows].reshape(2, 128)
    bias[2:4] = qk_b[k_rows].reshape(2, 128)
    bias[4:6] = v_b.reshape(2, 128)
    bias[6:8] = proj_b.reshape(2, 128)
    bias[8:12] = fc1_b.reshape(4, 128)
    bias[12:14] = fc2_b.reshape(2, 128)
    bias[14:16] = pe_b.reshape(2, 128)

    xn = x.reshape(B, C, HS, WS)
    in_maps = []
    for core in range(N_CORES):
        b, r = core // 4, core % 4
        xrot = np.roll(xn[b], -12 * r, axis=1)                 # rotate rows
        xb = np.ascontiguousarray(xrot.reshape(C, N).reshape(2, 128, N)).astype(BF16)
        xloc = np.ascontiguousarray(
            xrot[:, :12, :].reshape(C, NQ).reshape(2, 128, NQ)).astype(np.float32)
        mask = np.ones((18, 54), np.float32)
        if r == 0:
            mask[0:3, :] = 0.0                                  # top image border
        if r == 3:
            mask[15:18, :] = 0.0                                # bottom image border
        pemask = np.broadcast_to(mask.reshape(1, 972), (128, 972)).astype(BF16)
        in_maps.append({
            "xb": xb, "xloc": xloc, "pemask": np.ascontiguousarray(pemask),
            "wqkv": wqkv, "wvt": wvt, "wproj": wproj, "wfc1": wfc1, "wfc2": wfc2,
            "pew": pew, "bias": bias,
        })
    return in_maps


def kernel(**inputs) -> np.ndarray:
    nc = _get_nc()
    in_maps = _prep_inputs(inputs)
    res = run_bass_kernel_spmd(nc, in_maps, core_ids=list(range(N_CORES)),
                               trace=False)
    out = np.zeros((B, C, HS, WS), np.float32)
    for core in range(N_CORES):
        b, r = core // 4, core % 4
        y = res.results[core]["y"].reshape(C, 12, WS)
        out[b, :, 12 * r:12 * (r + 1), :] = y
    return out


def run_traced(inputs):
    """test-harness helper: run with NTFF tracing, return (out, results)."""
    nc = _get_nc()
    in_maps = _prep_inputs(inputs)
    res = run_bass_kernel_spmd(nc, in_maps, core_ids=list(range(N_CORES)),
                               trace=True)
    out = np.zeros((B, C, HS, WS), np.float32)
    for core in range(N_CORES):
        b, r = core // 4, core % 4
        y = res.results[core]["y"].reshape(C, 12, WS)
        out[b, :, 12 * r:12 * (r + 1), :] = y
    return out, res
